# revision 79
# baseline (speedup 1.0000x reference)
"""Trainium2 Bass kernel for nn_LocalSumMessageFunction (GNN message passing).

Strategy (node-sharded, SPMD over 8 cores):
  - Each core owns a contiguous 1/8 slice of the nodes. An "eval" is an
    (edge, port) pair, assigned to the core owning its *target* node. Host
    pre-sorts each core's evals by target node and packs them into bins:
    <=256 evals per port + <=128 distinct target nodes per bin. Two bins
    form a "pair" processed together (1024 eval columns). The irregular
    coordinate gather is resolved on the host (the runtime's indirect-DMA
    path is unusable), shipping a dense transposed coordinate stream.
  - The edge-feature input block and L1 bias are folded into that stream
    on the host: G is solved per port so G @ W1_c = [W1_ef; b1], and the
    kernel ships c' = c + [ef,1] @ G, making W1c^T c' = W1^T x + b1
    exactly. This removes the awkward K=17 matmul (small-K passes drop
    the PE into a 2x-cost 32-row tiling mode) AND a third of L1's full
    passes: L1 is just two K=128 matmuls per psum tile. G's rows are
    ill-conditioned (cond(W1_c) ~ 1e3) and amplify c' to ~14 sigma, so
    the c' stream and W1c use fp16 (10-bit mantissa); bf16 would fail
    the 2e-2 gate at 2.1e-2.
  - Device, per pair: 3-layer MLP on the tensor engine in feature-major
    orientation (weights stationary, eval columns streamed, fp32 psum).
    L2 bias+relu ride the scalar-engine activation; relus alternate
    between vector and scalar engines. L3 bias is added by the
    psum->sbuf message copy (vector scalar_tensor_tensor, split per bin
    so the bin-0 scatter starts early).
  - Scatter-add: per 128-eval chunk a one-hot (eval x slot) bf16 matrix
    is shipped from the host (masked-out evals get an all-zero row, which
    folds the non_fictitious mask in for free). One accumulation group of
    8 matmuls scatters both bins of a pair into a single [128, 256] PSUM
    accumulator; one tanh per pair (software-pipelined into the next
    pair's scalar-engine queue) stages the output, which the host
    scatters back to node rows (pure permutation).
  - Throughput plumbing: the ct stream (512KB/pair) is split across the
    sync and scalar DMA rings (one ring sustains ~87 GB/s = 5.9us for
    512KB, which would bound the ~4.8us tensor-limited pair period); oht
    rides the gpsimd ring; constants are spread across rings so the
    first pairs' inputs are not queued behind them.
"""

import numpy as np
import ml_dtypes

try:
    import concourse.bacc as bacc
except ImportError:  # pragma: no cover
    import sys

    sys.path.insert(0, "/opt/trn_rl_repo")
    import concourse.bacc as bacc

from concourse import mybir, tile
from concourse.bass_utils import run_bass_kernel_spmd

BF16 = ml_dtypes.bfloat16
AF = mybir.ActivationFunctionType
ALU = mybir.AluOpType

# Problem geometry (hardcoded per the harness contract).
N_NODES = 100000
N_EDGES = 250000
LATENT = 128
NF = 16
OUT = 128
D1 = 256  # hidden width
NCORES = 8

PORT_CAP = 256  # max evals per port per bin (2 chunks of 128)
NODE_CAP = 128  # max distinct target nodes per bin

# Engine for the relu after L1/L2, per (port, mt): 'dve' or 'act'.
# Alternating per mt makes each port's relu pair run concurrently on
# both engines, halving the L1->L2 and L2->L3 dependency latency.
RELU_L1_ENGINE = ("dve", "act", "dve", "act")
RELU_L2_ENGINE = ("act", "dve", "act", "dve")


def _pack_bins(cnt1, cnt2):
    """First-fit pack nodes into bins under the port/node caps.

    Nodes are visited in natural order but may backfill any open bin,
    which packs ~3% tighter than contiguous splitting. Returns a list of
    local-node-id arrays; a node's slot is its position in its bin.
    """
    n = len(cnt1)
    stats = []  # [p1, p2, nn] per bin
    members = []
    for i in range(n):
        r1 = int(cnt1[i])
        r2 = int(cnt2[i])
        if r1 + r2 == 0:
            continue
        for b, st in enumerate(stats):
            if st[0] + r1 <= PORT_CAP and st[1] + r2 <= PORT_CAP and st[2] < NODE_CAP:
                st[0] += r1
                st[1] += r2
                st[2] += 1
                members[b].append(i)
                break
        else:
            stats.append([r1, r2, 1])
            members.append([i])
    return [np.asarray(m, np.int64) for m in members]


def _prepare(inputs, ncores=NCORES, n_nodes=N_NODES):
    """Host-side sharding: build per-core in_maps + per-bin node lists."""
    npc = n_nodes // ncores
    a1 = np.asarray(inputs["addr_port1"]).astype(np.int64)
    a2 = np.asarray(inputs["addr_port2"]).astype(np.int64)
    ef = np.asarray(inputs["edge_features"], dtype=np.float32)
    mask = np.asarray(inputs["non_fictitious"], dtype=np.float32)
    coordsf = np.asarray(inputs["coordinates"], dtype=np.float32)

    per_core = []
    for k in range(ncores):
        n0, n1 = k * npc, (k + 1) * npc
        e1 = np.nonzero((a1 >= n0) & (a1 < n1))[0]
        e1 = e1[np.argsort(a1[e1], kind="stable")]
        e2 = np.nonzero((a2 >= n0) & (a2 < n1))[0]
        e2 = e2[np.argsort(a2[e2], kind="stable")]
        cnt1 = np.bincount(a1[e1] - n0, minlength=npc)
        cnt2 = np.bincount(a2[e2] - n0, minlength=npc)
        off1 = np.concatenate([[0], np.cumsum(cnt1)])
        off2 = np.concatenate([[0], np.cumsum(cnt2)])
        bins = _pack_bins(cnt1, cnt2)
        per_core.append((n0, e1, e2, cnt1, cnt2, off1, off2, bins))

    B = max(len(pc[7]) for pc in per_core)
    B = (B + 1) & ~1  # even
    S = B // 2

    # Constant (replicated) tensors, same for every core.
    w1c = []  # [128, 2, 256] per port: c1/c2 K-blocks
    w2 = []
    w3 = []
    # The edge-feature contribution is folded into the coordinate stream:
    # solve G so that G @ W1_c = [W1_ef; b1], then ship c' = c + [ef,1] @ G.
    # W1c^T c' = W1c^T c + W1ef^T ef + b1 exactly. G amplifies magnitudes
    # (cond(W1_c) ~ 1e3), so the c' stream and W1c use fp16 (10-bit mantissa)
    # instead of bf16.
    efb = np.concatenate([ef, np.ones((len(ef), 1), np.float32)], axis=1)
    addp = []  # per port: [n_edges, 256] fold-in term
    bcols = np.zeros((128, 4), np.float32)
    b3rep = np.zeros((128, 512), np.float32)
    for p, pre in enumerate(["p1", "p2"]):
        W1 = np.asarray(inputs[f"{pre}_W1"], np.float32)
        b1 = np.asarray(inputs[f"{pre}_b1"], np.float32)
        W2 = np.asarray(inputs[f"{pre}_W2"], np.float32)
        b2 = np.asarray(inputs[f"{pre}_b2"], np.float32)
        W3 = np.asarray(inputs[f"{pre}_W3"], np.float32)
        b3 = np.asarray(inputs[f"{pre}_b3"], np.float32)

        Wefb = np.concatenate([W1[0:NF], b1[None, :]], axis=0)  # [17, 256]
        G = np.linalg.solve(W1[NF:].T, Wefb.T).T  # [17, 256]
        addp.append(efb @ G)

        wc = np.zeros((128, 2, 256), np.float32)
        wc[:, 0, :] = W1[NF : NF + 128, :]
        wc[:, 1, :] = W1[NF + 128 : NF + 256, :]
        w1c.append(wc.astype(np.float16))

        w2.append(W2.astype(BF16))
        w3.append(W3.astype(BF16))
        for mt in (0, 1):
            bcols[:, 2 * p + mt] = b2[128 * mt : 128 * (mt + 1)]
        for rep in range(2):
            b3rep[:, 256 * p + 128 * rep : 256 * p + 128 * (rep + 1)] = b3[None, :]

    in_maps = []
    nodelists = []  # [core][bin] -> global node ids (slot order)
    for k in range(ncores):
        n0, e1, e2, cnt1, cnt2, off1, off2, bins = per_core[k]
        CT = np.zeros((S, 128, 2, 1024), np.float16)  # [pp, t(c1/c2), evalcol]
        OHM = np.zeros((S, 128, 2, 4, 128), BF16)  # [evalrow, bi, j, slot]
        nl_core = []
        slot_arr = np.full(npc, -1, np.int64)
        for b in range(B):
            s, bi = b // 2, b % 2
            if b >= len(bins):
                nl_core.append(np.zeros((0,), np.int64))
                continue
            nodes = bins[b]
            nl_core.append(nodes + n0)
            slot_arr[nodes] = np.arange(len(nodes))
            for port, (e, off, addr) in enumerate([(e1, off1, a1), (e2, off2, a2)]):
                eids = np.concatenate(
                    [e[off[i] : off[i + 1]] for i in nodes] or [np.zeros(0, np.int64)]
                )
                kk = len(eids)
                assert kk <= PORT_CAP
                idx = np.arange(kk)
                cols = 512 * port + 256 * bi + idx
                CT[s, :, 0, cols] = (coordsf[a1[eids]] + addp[port][eids, 0:128]).astype(np.float16)
                CT[s, :, 1, cols] = (coordsf[a2[eids]] + addp[port][eids, 128:256]).astype(np.float16)
                sl = slot_arr[addr[eids] - n0]
                valid = (sl >= 0) & (mask[eids] != 0.0)  # mask folds in here
                j = 2 * port + idx // 128
                OHM[s, (idx % 128)[valid], bi, j[valid], sl[valid]] = 1.0
            slot_arr[nodes] = -1
        nodelists.append(nl_core)

        # Weights packed into single tensors (one DMA each) so startup
        # isn't serialized on ~650ns-per-descriptor queue posts.
        wall = np.zeros((128, 1536), BF16)
        for p in (0, 1):
            for kt in (0, 1):
                wall[:, (2 * p + kt) * 256 : (2 * p + kt) * 256 + 256] = w2[p][kt * 128 : (kt + 1) * 128, :]
                wall[:, 1024 + (2 * p + kt) * 128 : 1024 + (2 * p + kt) * 128 + 128] = w3[p][kt * 128 : (kt + 1) * 128, :]
        w1call = np.stack([w1c[0], w1c[1]], axis=1)  # [128, 2, 2, 256] fp16
        im = {
            "ct": CT,
            "ohm": OHM,
            "bcols": bcols,
            "b3rep2": np.tile(b3rep, (1, 2)).astype(BF16),
            "zc": np.zeros((128, 8), BF16),
            "wall": wall,
            "w1call": w1call,
        }
        in_maps.append(im)
    return in_maps, nodelists, B


def _build(B, n_nodes=N_NODES):
    """Build the SPMD Bass program (one core's instruction stream)."""
    dt = mybir.dt
    nc = bacc.Bacc("TRN2", target_bir_lowering=False, debug=False)
    S = B // 2

    ct = nc.dram_tensor("ct", [S, 128, 2, 1024], dt.float16, kind="ExternalInput").ap()
    ohm = nc.dram_tensor("ohm", [S, 128, 2, 4, 128], dt.bfloat16, kind="ExternalInput").ap()
    wall = nc.dram_tensor("wall", [128, 1536], dt.bfloat16, kind="ExternalInput").ap()
    w1call = nc.dram_tensor("w1call", [128, 2, 2, 256], dt.float16, kind="ExternalInput").ap()
    bcols = nc.dram_tensor("bcols", [128, 4], dt.float32, kind="ExternalInput").ap()
    zc = nc.dram_tensor("zc", [128, 8], dt.bfloat16, kind="ExternalInput").ap()
    b3rep2 = nc.dram_tensor("b3rep2", [128, 1024], dt.bfloat16, kind="ExternalInput").ap()
    staged = nc.dram_tensor("staged", [S, 128, 256], dt.bfloat16, kind="ExternalOutput").ap()

    with tile.TileContext(nc) as tc:
        from contextlib import ExitStack

        with ExitStack() as ctx:
            cpool = ctx.enter_context(tc.tile_pool(name="const", bufs=1))
            iopool = ctx.enter_context(tc.tile_pool(name="io", bufs=2))
            h1pool = ctx.enter_context(tc.tile_pool(name="h1", bufs=2))
            h2pool = ctx.enter_context(tc.tile_pool(name="h2", bufs=2))
            mpool = ctx.enter_context(tc.tile_pool(name="msgs", bufs=2))
            stgpool = ctx.enter_context(tc.tile_pool(name="stg", bufs=2))
            mlppool = ctx.enter_context(tc.tile_pool(name="mlp", bufs=5, space="PSUM"))
            msgpool = ctx.enter_context(tc.tile_pool(name="msgp", bufs=1, space="PSUM"))
            accpool = ctx.enter_context(tc.tile_pool(name="accp", bufs=1, space="PSUM"))

            # Constant loads ride the scalar/gpsimd queues so the sync queue
            # can start streaming ct[0] immediately (startup is ring-BW bound).
            # b3rep goes first so the warmup burst can start after ~3us.
            def cload(shape, dtype, src, tag, eng):
                t = cpool.tile(shape, dtype, tag=tag, name=tag)
                eng.dma_start(out=t[:], in_=src)
                return t

            b3rep_t = cload([128, 1024], dt.bfloat16, b3rep2[:, :], "b3rep", nc.scalar)
            w1call_t = cload([128, 2, 2, 256], dt.float16, w1call[:], "w1call", nc.gpsimd)
            wall_t = cload([128, 1536], dt.bfloat16, wall[:, :], "wall", nc.gpsimd)
            w1c_t = [w1call_t[:, p] for p in (0, 1)]
            w2_t = [[wall_t[:, (2 * p + kt) * 256 : (2 * p + kt) * 256 + 256] for kt in (0, 1)] for p in (0, 1)]
            w3_t = [[wall_t[:, 1024 + (2 * p + kt) * 128 : 1024 + (2 * p + kt) * 128 + 128] for kt in (0, 1)] for p in (0, 1)]
            bcols_t = cload([128, 4], dt.float32, bcols[:, :], "bcols", nc.scalar)
            zc_t = cload([128, 8], dt.bfloat16, zc[:, :], "zc", nc.gpsimd)

            # PE warmup burst (~2.5us of dense matmuls to lift the p-state)
            wps = accpool.tile([128, 512], dt.float32, tag="acc", name="wps")
            for _ in range(12):
                nc.tensor.matmul(wps[:], lhsT=b3rep_t[:, 0:128], rhs=b3rep_t[:, 0:512], start=True, stop=True)

            pending = None  # (acc tile, s) awaiting tanh + staged DMA

            def flush_pending():
                nonlocal pending
                if pending is None:
                    return
                p_acc, p_s = pending
                stg = stgpool.tile([128, 256], dt.bfloat16, tag="stg", name="stg")
                nc.scalar.activation(stg[:], p_acc[:], AF.Tanh)
                # Output DMA on the scalar engine's queue: keeps the sync
                # queue exclusively feeding inputs (ct/eft/oht prefetch).
                nc.scalar.dma_start(out=staged[p_s], in_=stg[:])
                pending = None

            for s in range(S):
                # ct split across two DMA rings: a single ring at ~87 GB/s
                # would take 5.9us for the full 512KB and bound the period.
                ct_t = iopool.tile([128, 2, 1024], dt.float16, tag="ct")
                if s == 0:
                    # first pair: thirds across three rings to cut fill latency
                    nc.sync.dma_start(out=ct_t[:, 0, :], in_=ct[s][:, 0, :])
                    nc.scalar.dma_start(out=ct_t[:, 1, 0:512], in_=ct[s][:, 1, 0:512])
                    nc.gpsimd.dma_start(out=ct_t[:, 1, 512:1024], in_=ct[s][:, 1, 512:1024])
                else:
                    nc.sync.dma_start(out=ct_t[:, 0, :], in_=ct[s][:, 0, :])
                    nc.scalar.dma_start(out=ct_t[:, 1, :], in_=ct[s][:, 1, :])
                oht = iopool.tile([128, 2, 4, 128], dt.bfloat16, tag="oht")
                nc.gpsimd.dma_start(out=oht[:], in_=ohm[s])

                # --- L1 (ef + bias pre-folded into the fp16 c' stream) ---
                h1 = [h1pool.tile([128, 1024], dt.bfloat16, tag=f"h1_{mt}", name=f"h1_{mt}") for mt in (0, 1)]
                for p in (0, 1):
                    for mt in (0, 1):
                        msl = slice(128 * mt, 128 * (mt + 1))
                        cp = slice(512 * p, 512 * (p + 1))
                        ps = mlppool.tile([128, 512], dt.float32, tag="mlp", name=f"l1ps{p}{mt}")
                        nc.tensor.matmul(ps[:], lhsT=w1c_t[p][:, 0, msl], rhs=ct_t[:, 0, cp], start=True, stop=False)
                        nc.tensor.matmul(ps[:], lhsT=w1c_t[p][:, 1, msl], rhs=ct_t[:, 1, cp], start=False, stop=True)
                        if RELU_L1_ENGINE[2 * p + mt] == "dve":
                            nc.vector.tensor_scalar(
                                out=h1[mt][:, cp], in0=ps[:], scalar1=0.0, scalar2=None, op0=ALU.max
                            )
                        else:
                            nc.scalar.activation(h1[mt][:, cp], ps[:], AF.Relu)

                # Previous pair's tanh + output DMA: emitted here so the
                # scalar engine's wait on the (long since finished) scatter
                # doesn't block this pair's relus in its in-order queue.
                flush_pending()

                # --- L2 (bias + relu on the scalar engine) ---
                h2 = [h2pool.tile([128, 1024], dt.bfloat16, tag=f"h2_{mt}", name=f"h2_{mt}") for mt in (0, 1)]
                for p in (0, 1):
                    for mt in (0, 1):
                        msl = slice(128 * mt, 128 * (mt + 1))
                        cp = slice(512 * p, 512 * (p + 1))
                        ps = mlppool.tile([128, 512], dt.float32, tag="mlp", name=f"l2ps{p}{mt}")
                        nc.tensor.matmul(ps[:], lhsT=w2_t[p][0][:, msl], rhs=h1[0][:, cp], start=True, stop=False)
                        nc.tensor.matmul(ps[:], lhsT=w2_t[p][1][:, msl], rhs=h1[1][:, cp], start=False, stop=True)
                        if RELU_L2_ENGINE[2 * p + mt] == "dve":
                            nc.vector.tensor_scalar(
                                out=h2[mt][:, cp], in0=ps[:],
                                scalar1=bcols_t[:, 2 * p + mt : 2 * p + mt + 1], scalar2=0.0,
                                op0=ALU.add, op1=ALU.max,
                            )
                        else:
                            nc.scalar.activation(
                                h2[mt][:, cp], ps[:], AF.Relu,
                                bias=bcols_t[:, 2 * p + mt : 2 * p + mt + 1],
                            )

                # --- L3 msg (one wide 2-bank psum) + one-hot scatter ---
                # Each psum bank's accumulation group is opened by a dummy
                # zero-product 1-column matmul: a start=True matmul stalls the
                # PE until ALL in-flight matmuls complete, so pay that drain on
                # a cheap early instruction instead of the first real one.
                acc = accpool.tile([128, 256], dt.float32, tag="acc", name="acc")
                mps = msgpool.tile([128, 1024], dt.float32, tag="msgp", name="mps")
                for bi in (0, 1):
                    for j in range(4):
                        pj = j // 2
                        csl = slice(512 * pj + 256 * bi + 128 * (j % 2), 512 * pj + 256 * bi + 128 * (j % 2) + 128)
                        osl = slice(512 * bi + 128 * j, 512 * bi + 128 * (j + 1))
                        nc.tensor.matmul(mps[:, osl], lhsT=h2[0][:, csl], rhs=w3_t[pj][0][:], start=(j == 0), stop=False)
                        nc.tensor.matmul(mps[:, osl], lhsT=h2[1][:, csl], rhs=w3_t[pj][1][:], start=False, stop=(j == 3))
                # b3-add + bf16 conversion in two halves so the bin-0 scatter
                # can begin as soon as its half of the messages is staged.
                msgS = mpool.tile([128, 1024], dt.bfloat16, tag="msgS", name="msgS")
                for bi in (0, 1):
                    hsl = slice(512 * bi, 512 * (bi + 1))
                    nc.vector.scalar_tensor_tensor(
                        out=msgS[:, hsl], in0=mps[:, hsl], scalar=1.0, in1=b3rep_t[:, hsl], op0=ALU.mult, op1=ALU.add
                    )
                for bi in (0, 1):
                    for j in range(4):
                        nc.tensor.matmul(
                            acc[:, 128 * bi : 128 * (bi + 1)],
                            lhsT=oht[:, bi, j, :],
                            rhs=msgS[:, 512 * bi + 128 * j : 512 * bi + 128 * (j + 1)],
                            start=(bi == 0 and j == 0),
                            stop=(bi == 1 and j == 3),
                        )

                pending = (acc, s)
            flush_pending()

    nc.compile()
    return nc


def _assemble(results, nodelists, B, n_nodes=N_NODES):
    out = np.zeros((n_nodes, OUT), np.float32)
    for k, res in enumerate(results):
        st = res["staged"]
        for b in range(B):
            ids = nodelists[k][b]
            if len(ids):
                s, bi = b // 2, b % 2
                out[ids] = st[s, : len(ids), 128 * bi : 128 * bi + 128].astype(np.float32)
    return out


def kernel(**inputs):
    ncores = NCORES
    in_maps, nodelists, B = _prepare(inputs, ncores=ncores)
    nc = _build(B)
    res = run_bass_kernel_spmd(nc, in_maps, core_ids=list(range(ncores)))
    return _assemble(res.results, nodelists, B)



# revision 80
# speedup vs baseline: 1.0057x; 1.0057x over previous
"""Trainium2 Bass kernel for nn_LocalSumMessageFunction (GNN message passing).

Strategy (node-sharded, SPMD over 8 cores):
  - Each core owns a contiguous 1/8 slice of the nodes. An "eval" is an
    (edge, port) pair, assigned to the core owning its *target* node. Host
    pre-sorts each core's evals by target node and packs them into bins:
    <=256 evals per port + <=128 distinct target nodes per bin. Two bins
    form a "pair" processed together (1024 eval columns). The irregular
    coordinate gather is resolved on the host (the runtime's indirect-DMA
    path is unusable), shipping a dense transposed coordinate stream.
  - The edge-feature input block and L1 bias are folded into that stream
    on the host: G is solved per port so G @ W1_c = [W1_ef; b1], and the
    kernel ships c' = c + [ef,1] @ G, making W1c^T c' = W1^T x + b1
    exactly. This removes the awkward K=17 matmul (small-K passes drop
    the PE into a 2x-cost 32-row tiling mode) AND a third of L1's full
    passes: L1 is just two K=128 matmuls per psum tile. G's rows are
    ill-conditioned (cond(W1_c) ~ 1e3) and amplify c' to ~14 sigma, so
    the c' stream and W1c use fp16 (10-bit mantissa); bf16 would fail
    the 2e-2 gate at 2.1e-2.
  - Device, per pair: 3-layer MLP on the tensor engine in feature-major
    orientation (weights stationary, eval columns streamed, fp32 psum).
    L2 bias+relu ride the scalar-engine activation; relus alternate
    between vector and scalar engines. L3 bias is added by the
    psum->sbuf message copy (vector scalar_tensor_tensor, split per bin
    so the bin-0 scatter starts early).
  - Scatter-add: per 128-eval chunk a one-hot (eval x slot) bf16 matrix
    is shipped from the host (masked-out evals get an all-zero row, which
    folds the non_fictitious mask in for free). One accumulation group of
    8 matmuls scatters both bins of a pair into a single [128, 256] PSUM
    accumulator; one tanh per pair (software-pipelined into the next
    pair's scalar-engine queue) stages the output, which the host
    scatters back to node rows (pure permutation).
  - Throughput plumbing: the ct stream (512KB/pair) is split across the
    sync and scalar DMA rings (one ring sustains ~87 GB/s = 5.9us for
    512KB, which would bound the ~4.8us tensor-limited pair period); oht
    rides the gpsimd ring; constants are spread across rings so the
    first pairs' inputs are not queued behind them.
"""

import numpy as np
import ml_dtypes

try:
    import concourse.bacc as bacc
except ImportError:  # pragma: no cover
    import sys

    sys.path.insert(0, "/opt/trn_rl_repo")
    import concourse.bacc as bacc

from concourse import mybir, tile
from concourse.bass_utils import run_bass_kernel_spmd

BF16 = ml_dtypes.bfloat16
AF = mybir.ActivationFunctionType
ALU = mybir.AluOpType

# Problem geometry (hardcoded per the harness contract).
N_NODES = 100000
N_EDGES = 250000
LATENT = 128
NF = 16
OUT = 128
D1 = 256  # hidden width
NCORES = 8

PORT_CAP = 256  # max evals per port per bin (2 chunks of 128)
NODE_CAP = 128  # max distinct target nodes per bin

# Engine for the relu after L1/L2, per (port, mt): 'dve' or 'act'.
# Alternating per mt makes each port's relu pair run concurrently on
# both engines, halving the L1->L2 and L2->L3 dependency latency.
RELU_L1_ENGINE = ("dve", "act", "dve", "act")
RELU_L2_ENGINE = ("act", "dve", "act", "dve")


def _pack_bins(cnt1, cnt2):
    """First-fit pack nodes into bins under the port/node caps.

    Nodes are visited in natural order but may backfill any open bin,
    which packs ~3% tighter than contiguous splitting. Returns a list of
    local-node-id arrays; a node's slot is its position in its bin.
    """
    n = len(cnt1)
    stats = []  # [p1, p2, nn] per bin
    members = []
    for i in range(n):
        r1 = int(cnt1[i])
        r2 = int(cnt2[i])
        if r1 + r2 == 0:
            continue
        for b, st in enumerate(stats):
            if st[0] + r1 <= PORT_CAP and st[1] + r2 <= PORT_CAP and st[2] < NODE_CAP:
                st[0] += r1
                st[1] += r2
                st[2] += 1
                members[b].append(i)
                break
        else:
            stats.append([r1, r2, 1])
            members.append([i])
    return [np.asarray(m, np.int64) for m in members]


def _prepare(inputs, ncores=NCORES, n_nodes=N_NODES):
    """Host-side sharding: build per-core in_maps + per-bin node lists."""
    npc = n_nodes // ncores
    a1 = np.asarray(inputs["addr_port1"]).astype(np.int64)
    a2 = np.asarray(inputs["addr_port2"]).astype(np.int64)
    ef = np.asarray(inputs["edge_features"], dtype=np.float32)
    mask = np.asarray(inputs["non_fictitious"], dtype=np.float32)
    coordsf = np.asarray(inputs["coordinates"], dtype=np.float32)

    per_core = []
    for k in range(ncores):
        n0, n1 = k * npc, (k + 1) * npc
        e1 = np.nonzero((a1 >= n0) & (a1 < n1))[0]
        e1 = e1[np.argsort(a1[e1], kind="stable")]
        e2 = np.nonzero((a2 >= n0) & (a2 < n1))[0]
        e2 = e2[np.argsort(a2[e2], kind="stable")]
        cnt1 = np.bincount(a1[e1] - n0, minlength=npc)
        cnt2 = np.bincount(a2[e2] - n0, minlength=npc)
        off1 = np.concatenate([[0], np.cumsum(cnt1)])
        off2 = np.concatenate([[0], np.cumsum(cnt2)])
        bins = _pack_bins(cnt1, cnt2)
        per_core.append((n0, e1, e2, cnt1, cnt2, off1, off2, bins))

    B = max(len(pc[7]) for pc in per_core)
    B = (B + 1) & ~1  # even
    S = B // 2

    # Constant (replicated) tensors, same for every core.
    w1c = []  # [128, 2, 256] per port: c1/c2 K-blocks
    w2 = []
    w3 = []
    # The edge-feature contribution is folded into the coordinate stream:
    # solve G so that G @ W1_c = [W1_ef; b1], then ship c' = c + [ef,1] @ G.
    # W1c^T c' = W1c^T c + W1ef^T ef + b1 exactly. G amplifies magnitudes
    # (cond(W1_c) ~ 1e3), so the c' stream and W1c use fp16 (10-bit mantissa)
    # instead of bf16.
    efb = np.concatenate([ef, np.ones((len(ef), 1), np.float32)], axis=1)
    addp = []  # per port: [n_edges, 256] fold-in term
    bcols = np.zeros((128, 4), np.float32)
    b3rep = np.zeros((128, 512), np.float32)
    for p, pre in enumerate(["p1", "p2"]):
        W1 = np.asarray(inputs[f"{pre}_W1"], np.float32)
        b1 = np.asarray(inputs[f"{pre}_b1"], np.float32)
        W2 = np.asarray(inputs[f"{pre}_W2"], np.float32)
        b2 = np.asarray(inputs[f"{pre}_b2"], np.float32)
        W3 = np.asarray(inputs[f"{pre}_W3"], np.float32)
        b3 = np.asarray(inputs[f"{pre}_b3"], np.float32)

        Wefb = np.concatenate([W1[0:NF], b1[None, :]], axis=0)  # [17, 256]
        G = np.linalg.solve(W1[NF:].T, Wefb.T).T  # [17, 256]
        addp.append(efb @ G)

        wc = np.zeros((128, 2, 256), np.float32)
        wc[:, 0, :] = W1[NF : NF + 128, :]
        wc[:, 1, :] = W1[NF + 128 : NF + 256, :]
        w1c.append(wc.astype(np.float16))

        w2.append(W2.astype(BF16))
        w3.append(W3.astype(BF16))
        for mt in (0, 1):
            bcols[:, 2 * p + mt] = b2[128 * mt : 128 * (mt + 1)]
        for rep in range(2):
            b3rep[:, 256 * p + 128 * rep : 256 * p + 128 * (rep + 1)] = b3[None, :]

    in_maps = []
    nodelists = []  # [core][bin] -> global node ids (slot order)
    for k in range(ncores):
        n0, e1, e2, cnt1, cnt2, off1, off2, bins = per_core[k]
        CT = np.zeros((S, 128, 2, 1024), np.float16)  # [pp, t(c1/c2), evalcol]
        OHM = np.zeros((S, 128, 2, 4, 128), ml_dtypes.float8_e4m3)  # [evalrow, bi, j, slot]
        nl_core = []
        slot_arr = np.full(npc, -1, np.int64)
        for b in range(B):
            s, bi = b // 2, b % 2
            if b >= len(bins):
                nl_core.append(np.zeros((0,), np.int64))
                continue
            nodes = bins[b]
            nl_core.append(nodes + n0)
            slot_arr[nodes] = np.arange(len(nodes))
            for port, (e, off, addr) in enumerate([(e1, off1, a1), (e2, off2, a2)]):
                eids = np.concatenate(
                    [e[off[i] : off[i + 1]] for i in nodes] or [np.zeros(0, np.int64)]
                )
                kk = len(eids)
                assert kk <= PORT_CAP
                idx = np.arange(kk)
                cols = 512 * port + 256 * bi + idx
                CT[s, :, 0, cols] = (coordsf[a1[eids]] + addp[port][eids, 0:128]).astype(np.float16)
                CT[s, :, 1, cols] = (coordsf[a2[eids]] + addp[port][eids, 128:256]).astype(np.float16)
                sl = slot_arr[addr[eids] - n0]
                valid = (sl >= 0) & (mask[eids] != 0.0)  # mask folds in here
                j = 2 * port + idx // 128
                OHM[s, (idx % 128)[valid], bi, j[valid], sl[valid]] = 1.0
            slot_arr[nodes] = -1
        nodelists.append(nl_core)

        # Weights packed into single tensors (one DMA each) so startup
        # isn't serialized on ~650ns-per-descriptor queue posts.
        wall = np.zeros((128, 1536), BF16)
        for p in (0, 1):
            for kt in (0, 1):
                wall[:, (2 * p + kt) * 256 : (2 * p + kt) * 256 + 256] = w2[p][kt * 128 : (kt + 1) * 128, :]
                wall[:, 1024 + (2 * p + kt) * 128 : 1024 + (2 * p + kt) * 128 + 128] = w3[p][kt * 128 : (kt + 1) * 128, :]
        w1call = np.stack([w1c[0], w1c[1]], axis=1)  # [128, 2, 2, 256] fp16
        im = {
            "ct": CT,
            "ohm": OHM,
            "bcols": bcols,
            "b3rep2": np.tile(b3rep, (1, 2)).astype(BF16),
            "zc": np.zeros((128, 8), BF16),
            "wall": wall,
            "w1call": w1call,
        }
        in_maps.append(im)
    return in_maps, nodelists, B


def _build(B, n_nodes=N_NODES):
    """Build the SPMD Bass program (one core's instruction stream)."""
    dt = mybir.dt
    nc = bacc.Bacc("TRN2", target_bir_lowering=False, debug=False)
    S = B // 2

    ct = nc.dram_tensor("ct", [S, 128, 2, 1024], dt.float16, kind="ExternalInput").ap()
    ohm = nc.dram_tensor("ohm", [S, 128, 2, 4, 128], dt.float8e4, kind="ExternalInput").ap()
    wall = nc.dram_tensor("wall", [128, 1536], dt.bfloat16, kind="ExternalInput").ap()
    w1call = nc.dram_tensor("w1call", [128, 2, 2, 256], dt.float16, kind="ExternalInput").ap()
    bcols = nc.dram_tensor("bcols", [128, 4], dt.float32, kind="ExternalInput").ap()
    zc = nc.dram_tensor("zc", [128, 8], dt.bfloat16, kind="ExternalInput").ap()
    b3rep2 = nc.dram_tensor("b3rep2", [128, 1024], dt.bfloat16, kind="ExternalInput").ap()
    staged = nc.dram_tensor("staged", [S, 128, 256], dt.bfloat16, kind="ExternalOutput").ap()

    with tile.TileContext(nc) as tc:
        from contextlib import ExitStack

        with ExitStack() as ctx:
            cpool = ctx.enter_context(tc.tile_pool(name="const", bufs=1))
            iopool = ctx.enter_context(tc.tile_pool(name="io", bufs=2))
            h1pool = ctx.enter_context(tc.tile_pool(name="h1", bufs=2))
            h2pool = ctx.enter_context(tc.tile_pool(name="h2", bufs=2))
            mpool = ctx.enter_context(tc.tile_pool(name="msgs", bufs=2))
            stgpool = ctx.enter_context(tc.tile_pool(name="stg", bufs=2))
            mlppool = ctx.enter_context(tc.tile_pool(name="mlp", bufs=5, space="PSUM"))
            msgpool = ctx.enter_context(tc.tile_pool(name="msgp", bufs=1, space="PSUM"))
            accpool = ctx.enter_context(tc.tile_pool(name="accp", bufs=1, space="PSUM"))

            # Constant loads ride the scalar/gpsimd queues so the sync queue
            # can start streaming ct[0] immediately (startup is ring-BW bound).
            # b3rep goes first so the warmup burst can start after ~3us.
            def cload(shape, dtype, src, tag, eng):
                t = cpool.tile(shape, dtype, tag=tag, name=tag)
                eng.dma_start(out=t[:], in_=src)
                return t

            b3rep_t = cload([128, 1024], dt.bfloat16, b3rep2[:, :], "b3rep", nc.scalar)
            w1call_t = cload([128, 2, 2, 256], dt.float16, w1call[:], "w1call", nc.gpsimd)
            wall_t = cload([128, 1536], dt.bfloat16, wall[:, :], "wall", nc.gpsimd)
            w1c_t = [w1call_t[:, p] for p in (0, 1)]
            w2_t = [[wall_t[:, (2 * p + kt) * 256 : (2 * p + kt) * 256 + 256] for kt in (0, 1)] for p in (0, 1)]
            w3_t = [[wall_t[:, 1024 + (2 * p + kt) * 128 : 1024 + (2 * p + kt) * 128 + 128] for kt in (0, 1)] for p in (0, 1)]
            bcols_t = cload([128, 4], dt.float32, bcols[:, :], "bcols", nc.scalar)
            zc_t = cload([128, 8], dt.bfloat16, zc[:, :], "zc", nc.gpsimd)

            # PE warmup burst (~2.5us of dense matmuls to lift the p-state)
            wps = accpool.tile([128, 512], dt.float32, tag="acc", name="wps")
            for _ in range(12):
                nc.tensor.matmul(wps[:], lhsT=b3rep_t[:, 0:128], rhs=b3rep_t[:, 0:512], start=True, stop=True)

            pending = None  # (acc tile, s) awaiting tanh + staged DMA

            def flush_pending():
                nonlocal pending
                if pending is None:
                    return
                p_acc, p_s = pending
                stg = stgpool.tile([128, 256], dt.bfloat16, tag="stg", name="stg")
                nc.scalar.activation(stg[:], p_acc[:], AF.Tanh)
                # Output DMA on the scalar engine's queue: keeps the sync
                # queue exclusively feeding inputs (ct/eft/oht prefetch).
                nc.scalar.dma_start(out=staged[p_s], in_=stg[:])
                pending = None

            for s in range(S):
                # ct split across two DMA rings: a single ring at ~87 GB/s
                # would take 5.9us for the full 512KB and bound the period.
                ct_t = iopool.tile([128, 2, 1024], dt.float16, tag="ct")
                if s == 0:
                    # first pair: thirds across three rings to cut fill latency
                    nc.sync.dma_start(out=ct_t[:, 0, :], in_=ct[s][:, 0, :])
                    nc.scalar.dma_start(out=ct_t[:, 1, 0:512], in_=ct[s][:, 1, 0:512])
                    nc.gpsimd.dma_start(out=ct_t[:, 1, 512:1024], in_=ct[s][:, 1, 512:1024])
                else:
                    nc.sync.dma_start(out=ct_t[:, 0, :], in_=ct[s][:, 0, :])
                    nc.scalar.dma_start(out=ct_t[:, 1, :], in_=ct[s][:, 1, :])
                oht = iopool.tile([128, 2, 4, 128], dt.float8e4, tag="oht")
                nc.gpsimd.dma_start(out=oht[:], in_=ohm[s])

                # --- L1 (ef + bias pre-folded into the fp16 c' stream) ---
                h1 = [h1pool.tile([128, 1024], dt.bfloat16, tag=f"h1_{mt}", name=f"h1_{mt}") for mt in (0, 1)]
                for p in (0, 1):
                    for mt in (0, 1):
                        msl = slice(128 * mt, 128 * (mt + 1))
                        cp = slice(512 * p, 512 * (p + 1))
                        ps = mlppool.tile([128, 512], dt.float32, tag="mlp", name=f"l1ps{p}{mt}")
                        nc.tensor.matmul(ps[:], lhsT=w1c_t[p][:, 0, msl], rhs=ct_t[:, 0, cp], start=True, stop=False)
                        nc.tensor.matmul(ps[:], lhsT=w1c_t[p][:, 1, msl], rhs=ct_t[:, 1, cp], start=False, stop=True)
                        if RELU_L1_ENGINE[2 * p + mt] == "dve":
                            nc.vector.tensor_scalar(
                                out=h1[mt][:, cp], in0=ps[:], scalar1=0.0, scalar2=None, op0=ALU.max
                            )
                        else:
                            nc.scalar.activation(h1[mt][:, cp], ps[:], AF.Relu)

                # Previous pair's tanh + output DMA: emitted here so the
                # scalar engine's wait on the (long since finished) scatter
                # doesn't block this pair's relus in its in-order queue.
                flush_pending()

                # --- L2 (bias + relu on the scalar engine) ---
                h2 = [h2pool.tile([128, 1024], dt.bfloat16, tag=f"h2_{mt}", name=f"h2_{mt}") for mt in (0, 1)]
                for p in (0, 1):
                    for mt in (0, 1):
                        msl = slice(128 * mt, 128 * (mt + 1))
                        cp = slice(512 * p, 512 * (p + 1))
                        ps = mlppool.tile([128, 512], dt.float32, tag="mlp", name=f"l2ps{p}{mt}")
                        nc.tensor.matmul(ps[:], lhsT=w2_t[p][0][:, msl], rhs=h1[0][:, cp], start=True, stop=False)
                        nc.tensor.matmul(ps[:], lhsT=w2_t[p][1][:, msl], rhs=h1[1][:, cp], start=False, stop=True)
                        if RELU_L2_ENGINE[2 * p + mt] == "dve":
                            nc.vector.tensor_scalar(
                                out=h2[mt][:, cp], in0=ps[:],
                                scalar1=bcols_t[:, 2 * p + mt : 2 * p + mt + 1], scalar2=0.0,
                                op0=ALU.add, op1=ALU.max,
                            )
                        else:
                            nc.scalar.activation(
                                h2[mt][:, cp], ps[:], AF.Relu,
                                bias=bcols_t[:, 2 * p + mt : 2 * p + mt + 1],
                            )

                # --- L3 msg (one wide 2-bank psum) + one-hot scatter ---
                # Each psum bank's accumulation group is opened by a dummy
                # zero-product 1-column matmul: a start=True matmul stalls the
                # PE until ALL in-flight matmuls complete, so pay that drain on
                # a cheap early instruction instead of the first real one.
                acc = accpool.tile([128, 256], dt.float32, tag="acc", name="acc")
                mps = msgpool.tile([128, 1024], dt.float32, tag="msgp", name="mps")
                for bi in (0, 1):
                    for j in range(4):
                        pj = j // 2
                        csl = slice(512 * pj + 256 * bi + 128 * (j % 2), 512 * pj + 256 * bi + 128 * (j % 2) + 128)
                        osl = slice(512 * bi + 128 * j, 512 * bi + 128 * (j + 1))
                        nc.tensor.matmul(mps[:, osl], lhsT=h2[0][:, csl], rhs=w3_t[pj][0][:], start=(j == 0), stop=False)
                        nc.tensor.matmul(mps[:, osl], lhsT=h2[1][:, csl], rhs=w3_t[pj][1][:], start=False, stop=(j == 3))
                # b3-add + bf16 conversion in two halves so the bin-0 scatter
                # can begin as soon as its half of the messages is staged.
                msgS = mpool.tile([128, 8, 128], dt.float8e4, tag="msgS", name="msgS")
                for bi in (0, 1):
                    hsl = slice(512 * bi, 512 * (bi + 1))
                    nc.vector.scalar_tensor_tensor(
                        out=msgS[:, 4 * bi : 4 * bi + 4, :], in0=mps[:, hsl], scalar=1.0,
                        in1=b3rep_t[:, hsl], op0=ALU.mult, op1=ALU.add
                    )
                for bi in (0, 1):
                    for jp in (0, 1):
                        nc.tensor.matmul(
                            acc[:, 128 * bi : 128 * (bi + 1)],
                            lhsT=oht[:, bi, 2 * jp : 2 * jp + 2, :],
                            rhs=msgS[:, 4 * bi + 2 * jp : 4 * bi + 2 * jp + 2, :],
                            start=(bi == 0 and jp == 0),
                            stop=(bi == 1 and jp == 1),
                            perf_mode=mybir.MatmulPerfMode.DoubleRow,
                        )

                pending = (acc, s)
            flush_pending()

    nc.compile()
    return nc


def _assemble(results, nodelists, B, n_nodes=N_NODES):
    out = np.zeros((n_nodes, OUT), np.float32)
    for k, res in enumerate(results):
        st = res["staged"]
        for b in range(B):
            ids = nodelists[k][b]
            if len(ids):
                s, bi = b // 2, b % 2
                out[ids] = st[s, : len(ids), 128 * bi : 128 * bi + 128].astype(np.float32)
    return out


def kernel(**inputs):
    ncores = NCORES
    in_maps, nodelists, B = _prepare(inputs, ncores=ncores)
    nc = _build(B)
    res = run_bass_kernel_spmd(nc, in_maps, core_ids=list(range(ncores)))
    return _assemble(res.results, nodelists, B)

